# revision 14
# baseline (speedup 1.0000x reference)
"""Trainium2 Bass kernel for nn_GRUWithAttentionModel (B=4,D=60,S=512,F=158,C=64,H=128).

Sharding: phase 1 (per-day attention -> market vector) is sharded over the 240
(day,batch) pairs, 30 per core; the tiny (64,240) market matrix is AllGathered;
phase 2/3 (layernorm + GRU over days + head) is sharded over the 512 stocks,
64 per core (256 sequences/core).

Key ideas over the naive formulation:
- queries AND keys are compacted host-side: masked stocks contribute exactly
  0 to the market vector (query mask u=0; key mask exp(-2.5e8)=0), so each
  pair keeps only the first 384 mask-sorted stocks (max unmasked ~300) and
  Q/K/V all read the same compacted x tile.
- ctx is computed in [stock, C] orientation directly (no transposes); the
  softmax denominator rides along as a ones column of the V tiles.
- the LN rank-1 term (-mu*rstd x A) is folded into the LT1 matmul via an
  extra row of au1 (kills 6 rank-1 matmuls per day).
- GRU state stays split as h = t3 + zh (keeps the serial per-day dependency
  chain short); the n-gate r*(HN+bhh) runs as a cheap all-SBUF bf16 multiply
  against an off-chain HN+bhh evacuation.
- x-side prep is done 4 days at a time from flat rstd/-mu*rstd rows.

All bulk tensors are staged bf16 (inputs ~N(0,1), tolerance 2e-2); matmul
accumulation stays fp32 in PSUM.

Self-contained: call kernel(**inputs) with the full reference.setup_inputs()
arrays; returns the full (4,512,1) output.
"""
import numpy as np
import ml_dtypes

from concourse import bacc, tile, mybir
from concourse.bass import ts
from concourse.bass_utils import run_bass_kernel_spmd

F32 = mybir.dt.float32
BF16 = mybir.dt.bfloat16
BF = ml_dtypes.bfloat16

B, D, S, F, C, H, OUT = 4, 60, 512, 158, 64, 128, 1
LN_EPS = 1e-5
NCORE = 8
PPC = (B * D) // NCORE      # 30 pairs/core, pair p = d*4+b (d-major)
SL = S // NCORE             # 64 stocks/core
N = B * SL                  # 256 sequences/core
FC = F + C                  # 222
G3 = 3 * H                  # 384
F1 = F - 128                # 30 leftover feature rows
FA1 = F1 + 1                # +1 mask row (carries QKV biases for live stocks)
NK = 384                    # compacted (mask-sorted) stocks per pair
NKC = NK // 128             # 3 chunks


# ---------------------------------------------------------------- host prep
def host_prep(x, feature_mask, wq, bq, wk, bk, wv, bv, ln_g, ln_b,
              w_ih, w_hh, b_ih, b_hh, w1, b1, w2, b2):
    f32 = np.float32
    x = np.asarray(x, f32)
    x_att = x.transpose(1, 0, 2, 3).reshape(B * D, S, F)          # p = d*4+b
    mask_p = np.asarray(feature_mask, f32).transpose(1, 0, 2).reshape(B * D, S)
    denom = np.maximum(mask_p.sum(1), 1.0).astype(f32)
    u = (mask_p / denom[:, None]).astype(f32)

    # mask-sorted stock order per pair: unmasked first, truncate to NK
    order = np.argsort(~mask_p.astype(bool), axis=1, kind='stable')[:, :NK]
    assert mask_p.sum(1).max() <= NK
    x_key = np.take_along_axis(x_att, order[:, :, None], axis=1)  # (BD,NK,F)
    mask_key = np.take_along_axis(mask_p, order, axis=1)          # (BD,NK)
    u_key = np.take_along_axis(u, order, axis=1)                  # (BD,NK)
    mneg3 = ((1.0 - mask_key) * np.float32(-2e9)).astype(f32)

    wp = (np.asarray(w_ih, f32) * np.asarray(ln_g, f32)[None, :])
    A = wp.sum(1).astype(f32)
    Bb = (np.asarray(w_ih, f32) @ np.asarray(ln_b, f32) + np.asarray(b_ih, f32))
    Bb = Bb.copy()
    Bb[:2 * H] += np.asarray(b_hh, f32)[:2 * H]
    # LT1 rows: [market 64 | x-feats 128:158 (30) | pad 2 | A @96]
    # (Bb biases are applied at the activations, not in the matmul)
    LT0 = np.ascontiguousarray(wp.T[:128]).astype(BF)             # (128,384)
    LT1 = np.ascontiguousarray(np.concatenate(
        [wp.T[F:FC], wp.T[128:F], np.zeros((2, G3), f32), A[None]],
        0)).astype(BF)                                            # (97,384)
    BBC = np.ascontiguousarray(Bb.reshape(3, 128).T.copy())       # (128,3)

    bqkv = np.stack([np.asarray(bq, f32), np.asarray(bk, f32),
                     np.asarray(bv, f32)], 0).reshape(1, 192)    # [bq|bk|bv]

    per_core = []
    for i in range(NCORE):
        pi = slice(PPC * i, PPC * (i + 1))
        sl = slice(SL * i, SL * (i + 1))
        xs_sl = x[:, :, sl, :]
        xsum = (xs_sl.sum(-1) / np.float32(FC)).astype(f32)
        xsq = ((xs_sl * xs_sl).sum(-1) / np.float32(FC)).astype(f32)
        # compacted attention side: (158, PPC*NK), shared by Q, K and V
        xkey_T = np.ascontiguousarray(
            x_key[pi].transpose(2, 0, 1).reshape(F, PPC * NK)).astype(BF)
        # seq side: (158, D*N): col = d*256 + seq
        xseq_T = np.ascontiguousarray(
            xs_sl.transpose(3, 1, 2, 0).reshape(F, D * N)).astype(BF)
        # xmx rows 0:31 = [x-key 128:158, maskrow] (cols 0:PPC*NK);
        # rows 31:61 = x-seq 128:158 (cols 0:D*N)
        xk1 = np.concatenate(
            [xkey_T[128:F], mask_key[pi].reshape(1, PPC * NK).astype(BF)], 0)
        xmx = np.zeros((94, PPC * S), BF)
        xmx[0:FA1, :PPC * NK] = xk1
        xmx[64:94] = xseq_T[128:F]
        per_core.append(dict(
            xk0=xkey_T[0:128], xmx=np.ascontiguousarray(xmx),
            xd0=xseq_T[0:128],
            uT=np.ascontiguousarray(
                u_key[pi].reshape(PPC, NKC, 128).transpose(2, 0, 1)
                .reshape(128, PPC * NKC)),
            mnegT=np.ascontiguousarray(
                mneg3[pi].reshape(PPC, NKC, 128).transpose(2, 0, 1)
                .reshape(128, PPC * NKC)),
            xs=np.ascontiguousarray(xsum.transpose(1, 2, 0).reshape(D, N)),
            xq=np.ascontiguousarray(xsq.transpose(1, 2, 0).reshape(D, N)),
            LT0=LT0, LT1=LT1, BBC=BBC,
            WHH=np.ascontiguousarray(np.asarray(w_hh, f32).T).astype(BF),
            bhh_n=np.ascontiguousarray(np.asarray(b_hh, f32)[2 * H:][:, None]),
            WQKV0=np.ascontiguousarray(np.concatenate(
                [wq[:128], wk[:128], wv[:128]], 1).astype(f32)).astype(BF),
            WQKV1=np.ascontiguousarray(np.concatenate(
                [np.concatenate([wq[128:], wk[128:], wv[128:]], 1), bqkv],
                0).astype(f32)).astype(BF),                       # (31,192)
            W1=np.ascontiguousarray(np.asarray(w1, f32)).astype(BF),
            B1=np.ascontiguousarray(np.asarray(b1, f32)[:, None]),
            W2=np.ascontiguousarray(np.asarray(w2, f32)).astype(BF),
            c222v=np.full((C, 1), 1.0 / FC, f32),
            B2=np.ascontiguousarray(np.asarray(b2, f32)[None, :]),
            identb=np.eye(128, dtype=f32).astype(BF),
        ))
    return per_core


INPUT_SPECS = dict(
    xk0=((128, PPC * NK), BF16), xmx=((94, PPC * S), BF16),
    xd0=((128, D * N), BF16),
    uT=((128, PPC * NKC), F32), mnegT=((128, PPC * NKC), F32),
    xs=((D, N), F32), xq=((D, N), F32),
    LT0=((128, G3), BF16), LT1=((97, G3), BF16),
    BBC=((128, 3), F32),
    WHH=((H, G3), BF16), bhh_n=((H, 1), F32),
    WQKV0=((128, 192), BF16), WQKV1=((FA1, 192), BF16),
    W1=((H, C), BF16), B1=((C, 1), F32), W2=((C, 1), BF16), B2=((1, 1), F32),
    c222v=((C, 1), F32), identb=((128, 128), BF16),
)


# ---------------------------------------------------------------- program
def build_program():
    nc = bacc.Bacc("TRN2", target_bir_lowering=False, debug=False,
                   num_devices=NCORE)
    dram = {k: nc.dram_tensor(k, list(shp), dt, kind="ExternalInput").ap()
            for k, (shp, dt) in INPUT_SPECS.items()}
    yout = nc.dram_tensor("yout", [1, N], F32, kind="ExternalOutput").ap()
    AL = mybir.AluOpType
    AF = mybir.ActivationFunctionType

    with tile.TileContext(nc) as tc:
        with (
            nc.allow_low_precision(reason="bf16 staging within 2e-2 tolerance"),
            tc.tile_pool(name="const", bufs=1) as cp,
            tc.tile_pool(name="dram", bufs=1, space="DRAM") as dp,
        ):
            # ---- persistent tiles (weights + preloaded activations)
            cst = {}
            for k in INPUT_SPECS:
                shp, dt = INPUT_SPECS[k]
                cst[k] = cp.tile(list(shp), dt, tag=k, name=k)
            for k in ("WQKV0", "WQKV1", "mnegT", "uT"):
                nc.sync.dma_start(cst[k][:], dram[k])
            bounds = [0, 1, 3, 6, 10, 15, 21, 30]  # pair-index chunk edges
            for ci in range(len(bounds) - 1):
                klo, khi = bounds[ci] * NK, bounds[ci + 1] * NK
                nc.sync.dma_start(cst["xk0"][:, klo:khi],
                                  dram["xk0"][:, klo:khi])
                nc.sync.dma_start(cst["xmx"][0:FA1, klo:khi],
                                  dram["xmx"][0:FA1, klo:khi])
            for k in ("xd0", "xs", "xq", "LT0", "LT1", "WHH", "bhh_n",
                      "W1", "B1", "W2", "B2", "identb", "c222v", "BBC"):
                nc.sync.dma_start(cst[k][:], dram[k])
            nc.sync.dma_start(cst["xmx"][64:94, :], dram["xmx"][64:94, :])
            onesb = cp.tile([1, 1], F32, tag="onesb")
            nc.vector.memset(onesb[:], 1.0)
            epsc = cp.tile([D, 1], F32, tag="epsc")
            nc.vector.memset(epsc[:], LN_EPS)
            mcols = cp.tile([C, PPC], F32, tag="mcols")
            market = cp.tile([C, B * D], F32, tag="market")
            market_bf = cp.tile([C, B * D], BF16, tag="market_bf")
            summc = cp.tile([D, 4], F32, tag="summc")
            sumsqc = cp.tile([D, 4], F32, tag="sumsqc")
            rstd2 = cp.tile([D, N], BF16, tag="rstd2")
            rown2 = cp.tile([D, N], BF16, tag="rown2")
            rrowA = cp.tile([1, D * N], BF16, tag="rrowA")
            rrowB = cp.tile([1, D * N], BF16, tag="rrowB")

            # ================= phase 1: attention -> market columns
            with (
                tc.tile_pool(name="w1p", bufs=2) as wp,
                tc.tile_pool(name="ps1", bufs=1, space="PSUM") as ps,
            ):
                # pre-set the ones column of the V tiles (both pool bufs)
                for _ in range(2):
                    t = wp.tile([128, NKC, C + 1], BF16, tag="ve",
                                name="ve_init")
                    for kc in range(NKC):
                        nc.gpsimd.memset(t[:, kc, C:C + 1], 1.0)

                def stageA(p):
                    xk0 = cst["xk0"][:, ts(p, NK)]
                    xk1 = cst["xmx"][0:FA1, ts(p, NK)]
                    qkt = ps.tile([128, 512], F32, tag="qkt", name="qkt")
                    pq = qkt[0:64, 0:NK]
                    pk = qkt[64:128, 0:NK]
                    nc.tensor.matmul(pq, cst["WQKV0"][:, ts(0, C)],
                                     xk0, start=True, stop=False)
                    nc.tensor.matmul(pq, cst["WQKV1"][:, ts(0, C)],
                                     xk1, start=False, stop=True)
                    nc.tensor.matmul(pk, cst["WQKV0"][:, ts(1, C)],
                                     xk0, start=True, stop=False)
                    nc.tensor.matmul(pk, cst["WQKV1"][:, ts(1, C)],
                                     xk1, start=False, stop=True)
                    q_sb = wp.tile([C, NK], BF16, tag="q", name="q_sb")
                    nc.scalar.copy(q_sb[:], pq)
                    k_sb = wp.tile([C, NK], BF16, tag="k", name="k_sb")
                    nc.vector.tensor_copy(k_sb[:], pk)
                    # V chunks [128 stocks, C] (+ ones col set at pool init)
                    pv = ps.tile([128, NKC, C], F32, tag="pv", name="pv")
                    for kc in range(NKC):
                        nc.tensor.matmul(pv[:, kc, :], xk0[:, ts(kc, 128)],
                                         cst["WQKV0"][:, ts(2, C)],
                                         start=True, stop=False)
                        nc.tensor.matmul(pv[:, kc, :], xk1[:, ts(kc, 128)],
                                         cst["WQKV1"][:, ts(2, C)],
                                         start=False, stop=True)
                    ve = wp.tile([128, NKC, C + 1], BF16, tag="ve", name="ve")
                    nc.vector.tensor_copy(ve[:, :, 0:C], pv[:])
                    return q_sb, k_sb, ve

                def stageB(p, st):
                    q_sb, k_sb, ve = st
                    pss, eT = [], []
                    for c in range(NKC):
                        pt = ps.tile([128, NK], F32, tag=f"ss{c}", name="pss")
                        nc.tensor.matmul(pt[:], k_sb[:, ts(c, 128)],
                                         q_sb[:], start=True, stop=True)
                        pss.append(pt)
                    for c in range(NKC):
                        et = wp.tile([128, NK], BF16, tag=f"eT{c}", name="et")
                        nc.scalar.activation(
                            et[:], pss[c][:], AF.Exp, scale=0.125,
                            bias=cst["mnegT"][:, NKC * p + c:NKC * p + c + 1])
                        eT.append(et)
                    # ctx in [stock, C(+denom)] orientation, NKC stock chunks
                    cxm = ps.tile([128, NKC, C + 1], F32, tag="cxm",
                                  name="cxm")
                    for sc in range(NKC):
                        for tc_ in range(NKC):
                            nc.tensor.matmul(cxm[:, sc, :],
                                             eT[tc_][:, ts(sc, 128)],
                                             ve[:, tc_, :],
                                             start=(tc_ == 0),
                                             stop=(tc_ == NKC - 1))
                    rr = wp.tile([128, NKC], F32, tag="rr", name="rr")
                    nc.vector.reciprocal(rr[:], cxm[:, :, C])
                    gT = wp.tile([128, NKC], BF16, tag="gT", name="gT")
                    nc.vector.tensor_tensor(
                        out=gT[:], in0=cst["uT"][:, NKC * p:NKC * (p + 1)],
                        in1=rr[:], op=AL.mult)
                    cx2 = wp.tile([128, NKC, C], BF16, tag="cx2", name="cx2")
                    nc.vector.tensor_copy(cx2[:], cxm[:, :, 0:C])
                    psm = ps.tile([C, 1], F32, tag="psm", name="psm")
                    for sc in range(NKC):
                        nc.tensor.matmul(psm[:], cx2[:, sc, :],
                                         gT[:, sc:sc + 1],
                                         start=(sc == 0), stop=(sc == NKC - 1))
                    nc.vector.tensor_copy(mcols[:, p:p + 1], psm[:])

                st = stageA(0)
                for p in range(PPC):
                    nxt = stageA(p + 1) if p + 1 < PPC else None
                    stageB(p, st)
                    st = nxt

            # ================= collective: market_cols -> full market
            cin = dp.tile([C, PPC], F32)
            call = dp.tile([NCORE * C, PPC], F32)
            nc.sync.dma_start(cin[:], mcols[:])
            nc.gpsimd.collective_compute(
                "AllGather", mybir.AluOpType.bypass,
                replica_groups=[list(range(NCORE))],
                ins=[cin[:].opt()], outs=[call[:].opt()])
            call_v = call[:].rearrange("(blk c) j -> blk c j", blk=NCORE)
            nc.sync.dma_start(
                market[:].rearrange("c (blk j) -> c blk j", blk=NCORE),
                call_v.transpose([1, 0, 2]))

            # ================= phase 2/3: LN stats + GRU + head
            with (
                tc.tile_pool(name="w3p", bufs=2) as w3,
                tc.tile_pool(name="ps3", bufs=1, space="PSUM") as ps,
            ):
                nc.vector.tensor_copy(market_bf[:], market[:])
                # market sums (scaled by 1/222) -> (60,4) layout
                msq = w3.tile([C, B * D], F32, tag="msq")
                nc.scalar.square(msq[:], market[:])
                for src, dst in ((market, summc), (msq, sumsqc)):
                    pst = ps.tile([128, B * D + 8], F32, tag="pst", name="pst")
                    psum = pst[0:1, 0:B * D]
                    nc.tensor.matmul(psum, cst["c222v"][:], src[:],
                                     start=True, stop=True)
                    srow = w3.tile([1, B * D], F32, tag="srow")
                    nc.vector.tensor_copy(srow[:], psum)
                    # (1,240) row -> (60,4): per-b strided transpose matmuls
                    pmin = pst[0:D, B * D:B * D + 4]
                    srow_v = srow[:].rearrange("o (d b) -> o d b", b=4)
                    for b in range(4):
                        nc.tensor.matmul(pmin[:, b:b + 1],
                                         srow_v[0:1, :, b],
                                         onesb[0:1, 0:1],
                                         start=True, stop=True)
                    nc.vector.tensor_copy(dst[:], pmin)

                # LN statistics, in day-halves
                mu = w3.tile([D, N], F32, tag="mu")
                ms = w3.tile([D, N], F32, tag="ms")
                mu2 = w3.tile([D, N], F32, tag="mu2")
                var = w3.tile([D, N], F32, tag="var")
                std = w3.tile([D, N], F32, tag="std")
                rstd = w3.tile([D, N], F32, tag="rstd")
                dsplits = [(0, 32), (32, 60)]
                for g, (d0, d1) in enumerate(dsplits):
                    dd = slice(d0, d1)
                    nc.vector.tensor_tensor(
                        out=mu[dd, :].rearrange("p (s b) -> p s b", b=4),
                        in0=cst["xs"][dd, :].rearrange("p (s b) -> p s b", b=4),
                        in1=summc[dd, :].unsqueeze(1)
                            .broadcast_to([d1 - d0, SL, 4]),
                        op=AL.add)
                    nc.vector.tensor_tensor(
                        out=ms[dd, :].rearrange("p (s b) -> p s b", b=4),
                        in0=cst["xq"][dd, :].rearrange("p (s b) -> p s b", b=4),
                        in1=sumsqc[dd, :].unsqueeze(1)
                            .broadcast_to([d1 - d0, SL, 4]),
                        op=AL.add)
                    nc.vector.tensor_tensor(out=mu2[dd, :], in0=mu[dd, :],
                                            in1=mu[dd, :], op=AL.mult)
                    nc.vector.tensor_tensor(out=var[dd, :], in0=ms[dd, :],
                                            in1=mu2[dd, :], op=AL.subtract)
                    nc.scalar.activation(std[dd, :], var[dd, :], AF.Sqrt,
                                         bias=epsc[dd, :])
                    nc.vector.reciprocal(rstd[dd, :], std[dd, :])
                    nc.vector.tensor_copy(rstd2[dd, :], rstd[dd, :])
                    nc.vector.scalar_tensor_tensor(
                        out=rown2[dd, :], in0=rstd[dd, :], scalar=-1.0,
                        in1=mu[dd, :], op0=AL.mult, op1=AL.mult)
                rs_dram_a = dp.tile([D, N], BF16, name="rs_dram_a")
                rs_dram_b = dp.tile([D, N], BF16, name="rs_dram_b")
                for d0, d1 in dsplits:
                    dd = slice(d0, d1)
                    nc.sync.dma_start(rs_dram_a[dd, :], rstd2[dd, :])
                    nc.sync.dma_start(rs_dram_b[dd, :], rown2[dd, :])
                    nc.sync.dma_start(
                        rrowA[0:1, N * d0:N * d1],
                        rs_dram_a[dd, :].rearrange("p f -> () (p f)"))
                    nc.sync.dma_start(
                        rrowB[0:1, N * d0:N * d1],
                        rs_dram_b[dd, :].rearrange("p f -> () (p f)"))

                # ---- GRU over days (x-side prepped 4 days at a time)
                # h[k] = (t3_k, zh_k): h = t3 + zh, kept unsummed for WHH
                h = [[None, None], [None, None]]
                for k in range(2):
                    for q in range(2):
                        hz = w3.tile([H, N // 2], BF16, tag=f"h{k}{q}",
                                     name=f"h{k}{q}")
                        nc.vector.memset(hz[:], 0.0)
                        h[k][q] = hz
                # au1 rows 94:96 stay zero; row 96 = row2 (-mu*rstd)
                for _ in range(2):
                    t = w3.tile([97, 4 * N], BF16, tag="au1", name="au1_init")
                    nc.vector.memset(t[:], 0.0)

                def prep(d):
                    """prepare au0/au1 for days [d, d+4)."""
                    nd = min(4, D - d)
                    w = nd * N
                    psr = w3.tile([128, 4 * N], BF16, tag="psr", name="psr")
                    au0 = w3.tile([128, 4 * N], BF16, tag="au0", name="au0")
                    au1 = w3.tile([97, 4 * N], BF16, tag="au1", name="au1")
                    nc.gpsimd.partition_broadcast(
                        psr[:, 0:w], rrowA[0:1, N * d:N * d + w])
                    nc.sync.dma_start(au1[96:97, 0:w],
                                      rrowB[0:1, N * d:N * d + w])
                    nc.vector.tensor_tensor(out=au0[:, 0:w],
                                            in0=cst["xd0"][:, N * d:N * d + w],
                                            in1=psr[:, 0:w], op=AL.mult)
                    nc.vector.tensor_tensor(
                        out=au1[0:C, 0:w].rearrange(
                            "p (dd s b) -> p dd s b", dd=nd, b=4),
                        in0=market_bf[:, 4 * d:4 * (d + nd)].rearrange(
                            "c (dd b) -> c dd () b", dd=nd)
                            .broadcast_to([C, nd, SL, 4]),
                        in1=psr[0:C, 0:w].rearrange(
                            "p (dd s b) -> p dd s b", dd=nd, b=4),
                        op=AL.mult)
                    nc.vector.tensor_tensor(
                        out=au1[C:C + F1, 0:w],
                        in0=cst["xmx"][64:94, N * d:N * d + w],
                        in1=psr[C:C + F1, 0:w], op=AL.mult)
                    return au0, au1

                HF = N // 2  # column half: two independent chains
                cur = prep(0)
                nxt = None
                for d in range(D):
                    if d % 4 == 0 and d > 0:
                        cur = nxt
                    au0f, au1f = cur
                    au0 = au0f[:, ts(d % 4, N)]
                    au1 = au1f[:, ts(d % 4, N)]
                    RZ, XN = [None, None], [None, None]
                    for k in range(2):
                        RZ[k] = ps.tile([128, 2 * HF], F32, tag=f"RZ{k}",
                                        name=f"RZ{k}")
                        XNt = ps.tile([128, 512], F32, tag=f"XN{k}",
                                        name=f"XN{k}")
                        XN[k] = XNt[:, 0:HF]
                    HNp = ps.tile([128, 2 * HF], F32, tag="HNp", name="HNp")
                    # accumulation groups strictly sequenced per psum bank
                    for k in range(2):
                        cc = ts(k, HF)
                        for gi in range(2):  # r, z gates -> RZ[k] halves
                            reg = RZ[k][:, ts(gi, HF)]
                            nc.tensor.matmul(reg, cst["LT0"][:, ts(gi, 128)],
                                             au0[:, cc], start=True,
                                             stop=False)
                            nc.tensor.matmul(reg, cst["LT1"][:, ts(gi, 128)],
                                             au1[:, cc], start=False,
                                             stop=False)
                            nc.tensor.matmul(reg, cst["WHH"][:, ts(gi, 128)],
                                             h[k][1][:], start=False,
                                             stop=False)
                            nc.tensor.matmul(reg, cst["WHH"][:, ts(gi, 128)],
                                             h[k][0][:], start=False,
                                             stop=True)
                        nc.tensor.matmul(XN[k], cst["LT0"][:, ts(2, 128)],
                                         au0[:, cc], start=True, stop=False)
                        nc.tensor.matmul(XN[k], cst["LT1"][:, ts(2, 128)],
                                         au1[:, cc], start=False, stop=False)
                        hreg = HNp[:, ts(k, HF)]
                        nc.tensor.matmul(hreg, cst["WHH"][:, ts(2, 128)],
                                         h[k][1][:], start=True, stop=False)
                        nc.tensor.matmul(hreg, cst["WHH"][:, ts(2, 128)],
                                         h[k][0][:], start=False, stop=True)
                    r_sb, z_sb, hnb, t1, zc, hs, zh, n_sb, t3 = (
                        [None, None] for _ in range(9))
                    for k in range(2):
                        r_sb[k] = w3.tile([H, HF], BF16, tag=f"r{k}",
                                          name=f"r{k}")
                        nc.scalar.activation(r_sb[k][:], RZ[k][:, 0:HF],
                                             AF.Sigmoid,
                                             bias=cst["BBC"][:, 0:1])
                    for k in range(2):
                        z_sb[k] = w3.tile([H, HF], BF16, tag=f"z{k}",
                                          name=f"z{k}")
                        nc.scalar.activation(z_sb[k][:], RZ[k][:, HF:2 * HF],
                                             AF.Sigmoid,
                                             bias=cst["BBC"][:, 1:2])
                        hnb[k] = w3.tile([H, HF], BF16, tag=f"hnb{k}",
                                         name=f"hnb{k}")
                        nc.scalar.activation(hnb[k][:], HNp[:, ts(k, HF)],
                                             AF.Identity,
                                             bias=cst["bhh_n"][:])
                    for k in range(2):
                        t1[k] = w3.tile([H, HF], BF16, tag=f"t1{k}",
                                        name=f"t1{k}")
                        nc.vector.tensor_tensor(out=t1[k][:], in0=r_sb[k][:],
                                                in1=hnb[k][:], op=AL.mult)
                        nc.tensor.matmul(XN[k], cst["identb"][:],
                                         t1[k][:], start=False, stop=True)
                        zc[k] = w3.tile([H, HF], BF16, tag=f"zc{k}",
                                        name=f"zc{k}")
                        nc.gpsimd.tensor_scalar(out=zc[k][:], in0=z_sb[k][:],
                                                scalar1=-1.0, scalar2=1.0,
                                                op0=AL.mult, op1=AL.add)
                        hs[k] = w3.tile([H, HF], BF16, tag=f"hs{k}",
                                        name=f"hs{k}")
                        nc.vector.tensor_tensor(out=hs[k][:], in0=h[k][0][:],
                                                in1=h[k][1][:], op=AL.add)
                        zh[k] = w3.tile([H, HF], BF16, tag=f"zh{k}",
                                        name=f"zh{k}")
                        nc.vector.tensor_tensor(out=zh[k][:], in0=z_sb[k][:],
                                                in1=hs[k][:], op=AL.mult)
                    if d + 1 < D and (d + 1) % 4 == 0:
                        nxt = prep(d + 1)
                    for k in range(2):
                        n_sb[k] = w3.tile([H, HF], BF16, tag=f"n{k}",
                                          name=f"n{k}")
                        nc.scalar.activation(n_sb[k][:], XN[k], AF.Tanh,
                                             bias=cst["BBC"][:, 2:3])
                        t3[k] = w3.tile([H, HF], BF16, tag=f"t3{k}",
                                        name=f"t3{k}")
                        nc.vector.tensor_tensor(out=t3[k][:], in0=n_sb[k][:],
                                                in1=zc[k][:], op=AL.mult)
                    h = [(t3[0], zh[0]), (t3[1], zh[1])]

                # ---- head
                phdt = ps.tile([128, 2 * HF], F32, tag="RZ0")
                phd = phdt[0:C, :]
                for k in range(2):
                    nc.tensor.matmul(phd[:, ts(k, HF)], cst["W1"][:],
                                     h[k][0][:], start=True, stop=False)
                    nc.tensor.matmul(phd[:, ts(k, HF)], cst["W1"][:],
                                     h[k][1][:], start=False, stop=True)
                hid = w3.tile([C, N], BF16, tag="hid")
                nc.scalar.activation(hid[:], phd, AF.Relu,
                                     bias=cst["B1"][:])
                psot = ps.tile([128, 2 * HF], F32, tag="HNp")
                pso = psot[0:1, 0:N]
                nc.tensor.matmul(pso, cst["W2"][:], hid[:],
                                 start=True, stop=True)
                yo = w3.tile([1, N], F32, tag="yo")
                nc.scalar.activation(yo[:], pso, AF.Identity,
                                     bias=cst["B2"][0:1, 0:1])
                nc.sync.dma_start(yout, yo[:])

    nc.compile()
    return nc


_NC_CACHE = None


def kernel(**inputs):
    global _NC_CACHE
    per_core = host_prep(**inputs)
    if _NC_CACHE is None:
        _NC_CACHE = build_program()
    nc = _NC_CACHE
    in_maps = [{k: pc[k] for k in INPUT_SPECS} for pc in per_core]
    res = run_bass_kernel_spmd(nc, in_maps, list(range(NCORE)))
    out = np.zeros((B, S, OUT), np.float32)
    for i in range(NCORE):
        out[:, SL * i:SL * (i + 1), 0] = (
            res.results[i]["yout"].reshape(SL, B).T)
    return out


# revision 17
# speedup vs baseline: 1.1194x; 1.1194x over previous
"""Trainium2 Bass kernel for nn_GRUWithAttentionModel (B=4,D=60,S=512,F=158,C=64,H=128).

Sharding: phase 1 (per-day attention -> market vector) is sharded over the 240
(day,batch) pairs, 30 per core; the tiny (64,240) market matrix is AllGathered;
phase 2/3 (layernorm + GRU over days + head) is sharded over the 512 stocks,
64 per core (256 sequences/core).

Key ideas over the naive formulation:
- queries AND keys are compacted host-side: masked stocks contribute exactly
  0 to the market vector (query mask u=0; key mask exp(-2.5e8)=0), so each
  pair keeps only the first 384 mask-sorted stocks (max unmasked ~300) and
  Q/K/V all read the same compacted x tile.
- ctx is computed in [stock, C] orientation directly (no transposes); the
  softmax denominator rides along as a ones column of the V tiles.
- the LN rank-1 term (-mu*rstd x A) is folded into the LT1 matmul via an
  extra row of au1 (kills 6 rank-1 matmuls per day).
- GRU state stays split as h = t3 + zh (keeps the serial per-day dependency
  chain short); the n-gate r*(HN+bhh) runs as a cheap all-SBUF bf16 multiply
  against an off-chain HN+bhh evacuation.
- x-side prep is done 4 days at a time from flat rstd/-mu*rstd rows.

All bulk tensors are staged bf16 (inputs ~N(0,1), tolerance 2e-2); matmul
accumulation stays fp32 in PSUM.

Self-contained: call kernel(**inputs) with the full reference.setup_inputs()
arrays; returns the full (4,512,1) output.
"""
import numpy as np
import ml_dtypes

from concourse import bacc, tile, mybir
from concourse.bass import ts
from concourse.bass_utils import run_bass_kernel_spmd

F32 = mybir.dt.float32
BF16 = mybir.dt.bfloat16
BF = ml_dtypes.bfloat16

B, D, S, F, C, H, OUT = 4, 60, 512, 158, 64, 128, 1
LN_EPS = 1e-5
NCORE = 8
PPC = (B * D) // NCORE      # 30 pairs/core, pair p = d*4+b (d-major)
SL = S // NCORE             # 64 stocks/core
N = B * SL                  # 256 sequences/core
FC = F + C                  # 222
G3 = 3 * H                  # 384
F1 = F - 128                # 30 leftover feature rows
FA1 = F1 + 1                # +1 mask row (carries QKV biases for live stocks)
NK = 384                    # compacted (mask-sorted) stocks per pair
NKC = NK // 128             # 3 chunks


# ---------------------------------------------------------------- host prep
def host_prep(x, feature_mask, wq, bq, wk, bk, wv, bv, ln_g, ln_b,
              w_ih, w_hh, b_ih, b_hh, w1, b1, w2, b2):
    f32 = np.float32
    x = np.asarray(x, f32)
    x_att = x.transpose(1, 0, 2, 3).reshape(B * D, S, F)          # p = d*4+b
    mask_p = np.asarray(feature_mask, f32).transpose(1, 0, 2).reshape(B * D, S)
    denom = np.maximum(mask_p.sum(1), 1.0).astype(f32)
    u = (mask_p / denom[:, None]).astype(f32)

    # mask-sorted stock order per pair: unmasked first, truncate to NK
    order = np.argsort(~mask_p.astype(bool), axis=1, kind='stable')[:, :NK]
    assert mask_p.sum(1).max() <= NK
    x_key = np.take_along_axis(x_att, order[:, :, None], axis=1)  # (BD,NK,F)
    mask_key = np.take_along_axis(mask_p, order, axis=1)          # (BD,NK)
    u_key = np.take_along_axis(u, order, axis=1)                  # (BD,NK)
    mneg3 = ((1.0 - mask_key) * np.float32(-2e9)).astype(f32)

    wp = (np.asarray(w_ih, f32) * np.asarray(ln_g, f32)[None, :])
    A = wp.sum(1).astype(f32)
    Bb = (np.asarray(w_ih, f32) @ np.asarray(ln_b, f32) + np.asarray(b_ih, f32))
    Bb = Bb.copy()
    Bb[:2 * H] += np.asarray(b_hh, f32)[:2 * H]
    # LT1 rows: [market 64 | x-feats 128:158 (30) | pad 2 | A @96]
    # (Bb biases are applied at the activations, not in the matmul)
    LT0 = np.ascontiguousarray(wp.T[:128]).astype(BF)             # (128,384)
    LT1 = np.ascontiguousarray(np.concatenate(
        [wp.T[F:FC], wp.T[128:F], np.zeros((2, G3), f32), A[None]],
        0)).astype(BF)                                            # (97,384)
    BBC = np.ascontiguousarray(Bb.reshape(3, 128).T.copy())       # (128,3)

    bqkv = np.stack([np.asarray(bq, f32), np.asarray(bk, f32),
                     np.asarray(bv, f32)], 0).reshape(1, 192)    # [bq|bk|bv]

    per_core = []
    for i in range(NCORE):
        pi = slice(PPC * i, PPC * (i + 1))
        sl = slice(SL * i, SL * (i + 1))
        xs_sl = x[:, :, sl, :]
        xsum = (xs_sl.sum(-1) / np.float32(FC)).astype(f32)
        xsq = ((xs_sl * xs_sl).sum(-1) / np.float32(FC)).astype(f32)
        # compacted attention side: (158, PPC*NK), shared by Q, K and V
        xkey_T = np.ascontiguousarray(
            x_key[pi].transpose(2, 0, 1).reshape(F, PPC * NK)).astype(BF)
        # seq side: (158, D*N): col = d*256 + seq
        xseq_T = np.ascontiguousarray(
            xs_sl.transpose(3, 1, 2, 0).reshape(F, D * N)).astype(BF)
        # xmx rows 0:31 = [x-key 128:158, maskrow] (cols 0:PPC*NK);
        # rows 31:61 = x-seq 128:158 (cols 0:D*N)
        xk1 = np.concatenate(
            [xkey_T[128:F], mask_key[pi].reshape(1, PPC * NK).astype(BF)], 0)
        xmx = np.zeros((94, PPC * S), BF)
        xmx[0:FA1, :PPC * NK] = xk1
        xmx[64:94] = xseq_T[128:F]
        per_core.append(dict(
            xk0=xkey_T[0:128], xmx=np.ascontiguousarray(xmx),
            xd0=xseq_T[0:128],
            uT=np.ascontiguousarray(
                u_key[pi].reshape(PPC, NKC, 128).transpose(2, 0, 1)
                .reshape(128, PPC * NKC)),
            mnegT=np.ascontiguousarray(
                mneg3[pi].reshape(PPC, NKC, 128).transpose(2, 0, 1)
                .reshape(128, PPC * NKC)),
            xs=np.ascontiguousarray(xsum.transpose(1, 2, 0).reshape(D, N)),
            xq=np.ascontiguousarray(xsq.transpose(1, 2, 0).reshape(D, N)),
            LT0=LT0, LT1=LT1, BBC=BBC,
            WHH=np.ascontiguousarray(np.asarray(w_hh, f32).T).astype(BF),
            bhh_n=np.ascontiguousarray(np.asarray(b_hh, f32)[2 * H:][:, None]),
            WQKV0=np.ascontiguousarray(np.concatenate(
                [wq[:128], wk[:128], wv[:128]], 1).astype(f32)).astype(BF),
            WQKV1=np.ascontiguousarray(np.concatenate(
                [np.concatenate([wq[128:], wk[128:], wv[128:]], 1), bqkv],
                0).astype(f32)).astype(BF),                       # (31,192)
            W1=np.ascontiguousarray(np.asarray(w1, f32)).astype(BF),
            B1=np.ascontiguousarray(np.asarray(b1, f32)[:, None]),
            W2=np.ascontiguousarray(np.asarray(w2, f32)).astype(BF),
            c222v=np.full((C, 1), 1.0 / FC, f32),
            B2=np.ascontiguousarray(np.asarray(b2, f32)[None, :]),
            identb=np.eye(128, dtype=f32).astype(BF),
        ))
    return per_core


INPUT_SPECS = dict(
    xk0=((128, PPC * NK), BF16), xmx=((94, PPC * S), BF16),
    xd0=((128, D * N), BF16),
    uT=((128, PPC * NKC), F32), mnegT=((128, PPC * NKC), F32),
    xs=((D, N), F32), xq=((D, N), F32),
    LT0=((128, G3), BF16), LT1=((97, G3), BF16),
    BBC=((128, 3), F32),
    WHH=((H, G3), BF16), bhh_n=((H, 1), F32),
    WQKV0=((128, 192), BF16), WQKV1=((FA1, 192), BF16),
    W1=((H, C), BF16), B1=((C, 1), F32), W2=((C, 1), BF16), B2=((1, 1), F32),
    c222v=((C, 1), F32), identb=((128, 128), BF16),
)


# ---------------------------------------------------------------- program
def build_program():
    nc = bacc.Bacc("TRN2", target_bir_lowering=False, debug=False,
                   num_devices=NCORE)
    dram = {k: nc.dram_tensor(k, list(shp), dt, kind="ExternalInput").ap()
            for k, (shp, dt) in INPUT_SPECS.items()}
    yout = nc.dram_tensor("yout", [1, N], F32, kind="ExternalOutput").ap()
    AL = mybir.AluOpType
    AF = mybir.ActivationFunctionType

    with tile.TileContext(nc) as tc:
        with (
            nc.allow_low_precision(reason="bf16 staging within 2e-2 tolerance"),
            tc.tile_pool(name="const", bufs=1) as cp,
            tc.tile_pool(name="dram", bufs=1, space="DRAM") as dp,
        ):
            # ---- persistent tiles (weights + preloaded activations)
            cst = {}
            for k in INPUT_SPECS:
                shp, dt = INPUT_SPECS[k]
                cst[k] = cp.tile(list(shp), dt, tag=k, name=k)
            for k in ("WQKV0", "WQKV1", "mnegT", "uT"):
                nc.sync.dma_start(cst[k][:], dram[k])
            bounds = [0, 1, 3, 6, 10, 15, 21, 30]  # pair-index chunk edges
            for ci in range(len(bounds) - 1):
                klo, khi = bounds[ci] * NK, bounds[ci + 1] * NK
                nc.sync.dma_start(cst["xk0"][:, klo:khi],
                                  dram["xk0"][:, klo:khi])
                nc.sync.dma_start(cst["xmx"][0:FA1, klo:khi],
                                  dram["xmx"][0:FA1, klo:khi])
            for k in ("xd0", "xs", "xq", "LT0", "LT1", "WHH", "bhh_n",
                      "W1", "B1", "W2", "B2", "identb", "c222v", "BBC"):
                nc.sync.dma_start(cst[k][:], dram[k])
            nc.sync.dma_start(cst["xmx"][64:94, :], dram["xmx"][64:94, :])
            onesb = cp.tile([1, 1], F32, tag="onesb")
            nc.vector.memset(onesb[:], 1.0)
            epsc = cp.tile([D, 1], F32, tag="epsc")
            nc.vector.memset(epsc[:], LN_EPS)
            mcols = cp.tile([C, PPC], F32, tag="mcols")
            market = cp.tile([C, B * D], F32, tag="market")
            market_bf = cp.tile([C, B * D], BF16, tag="market_bf")
            summc = cp.tile([D, 4], F32, tag="summc")
            sumsqc = cp.tile([D, 4], F32, tag="sumsqc")
            rstd2 = cp.tile([D, N], BF16, tag="rstd2")
            rown2 = cp.tile([D, N], BF16, tag="rown2")
            rrowA = cp.tile([1, D * N], BF16, tag="rrowA")
            rrowB = cp.tile([1, D * N], BF16, tag="rrowB")

            # ================= phase 1: attention -> market columns
            with (
                tc.tile_pool(name="w1p", bufs=2) as wp,
                tc.tile_pool(name="ps1", bufs=1, space="PSUM") as ps,
            ):
                # pre-set the ones column of the V tiles (both pool bufs)
                for _ in range(2):
                    t = wp.tile([128, NKC, C + 1], BF16, tag="ve",
                                name="ve_init")
                    for kc in range(NKC):
                        nc.gpsimd.memset(t[:, kc, C:C + 1], 1.0)

                def stageA(p):
                    xk0 = cst["xk0"][:, ts(p, NK)]
                    xk1 = cst["xmx"][0:FA1, ts(p, NK)]
                    qkt = ps.tile([128, 512], F32, tag="qkt", name="qkt")
                    pq = qkt[0:64, 0:NK]
                    pk = qkt[64:128, 0:NK]
                    nc.tensor.matmul(pq, cst["WQKV0"][:, ts(0, C)],
                                     xk0, start=True, stop=False)
                    nc.tensor.matmul(pq, cst["WQKV1"][:, ts(0, C)],
                                     xk1, start=False, stop=True)
                    nc.tensor.matmul(pk, cst["WQKV0"][:, ts(1, C)],
                                     xk0, start=True, stop=False)
                    nc.tensor.matmul(pk, cst["WQKV1"][:, ts(1, C)],
                                     xk1, start=False, stop=True)
                    q_sb = wp.tile([C, NK], BF16, tag="q", name="q_sb")
                    nc.vector.tensor_copy(q_sb[:], pq)
                    k_sb = wp.tile([C, NK], BF16, tag="k", name="k_sb")
                    nc.vector.tensor_copy(k_sb[:], pk)
                    # V chunks [128 stocks, C] (+ ones col set at pool init)
                    pv = ps.tile([128, NKC, C], F32, tag="pv", name="pv")
                    for kc in range(NKC):
                        nc.tensor.matmul(pv[:, kc, :], xk0[:, ts(kc, 128)],
                                         cst["WQKV0"][:, ts(2, C)],
                                         start=True, stop=False)
                        nc.tensor.matmul(pv[:, kc, :], xk1[:, ts(kc, 128)],
                                         cst["WQKV1"][:, ts(2, C)],
                                         start=False, stop=True)
                    ve = wp.tile([128, NKC, C + 1], BF16, tag="ve", name="ve")
                    nc.vector.tensor_copy(ve[:, :, 0:C], pv[:])
                    return q_sb, k_sb, ve

                def stageB(p, st):
                    q_sb, k_sb, ve = st
                    pss, eT = [], []
                    for c in range(NKC):
                        pt = ps.tile([128, NK], F32, tag=f"ss{c}", name="pss")
                        nc.tensor.matmul(pt[:], k_sb[:, ts(c, 128)],
                                         q_sb[:], start=True, stop=True)
                        pss.append(pt)
                    for c in range(NKC):
                        et = wp.tile([128, NK], BF16, tag=f"eT{c}", name="et")
                        nc.scalar.activation(
                            et[:], pss[c][:], AF.Exp, scale=0.125,
                            bias=cst["mnegT"][:, NKC * p + c:NKC * p + c + 1])
                        eT.append(et)
                    # ctx in [stock, C(+denom)] orientation, NKC stock chunks
                    cxm = ps.tile([128, NKC, C + 1], F32, tag="cxm",
                                  name="cxm")
                    for sc in range(NKC):
                        for tc_ in range(NKC):
                            nc.tensor.matmul(cxm[:, sc, :],
                                             eT[tc_][:, ts(sc, 128)],
                                             ve[:, tc_, :],
                                             start=(tc_ == 0),
                                             stop=(tc_ == NKC - 1))
                    rr = wp.tile([128, NKC], F32, tag="rr", name="rr")
                    nc.vector.reciprocal(rr[:], cxm[:, :, C])
                    gT = wp.tile([128, NKC], BF16, tag="gT", name="gT")
                    nc.vector.tensor_tensor(
                        out=gT[:], in0=cst["uT"][:, NKC * p:NKC * (p + 1)],
                        in1=rr[:], op=AL.mult)
                    cx2 = wp.tile([128, NKC, C], BF16, tag="cx2", name="cx2")
                    nc.vector.tensor_copy(cx2[:], cxm[:, :, 0:C])
                    psm = ps.tile([C, 1], F32, tag="psm", name="psm")
                    for sc in range(NKC):
                        nc.tensor.matmul(psm[:], cx2[:, sc, :],
                                         gT[:, sc:sc + 1],
                                         start=(sc == 0), stop=(sc == NKC - 1))
                    nc.vector.tensor_copy(mcols[:, p:p + 1], psm[:])

                st = stageA(0)
                for p in range(PPC):
                    nxt = stageA(p + 1) if p + 1 < PPC else None
                    stageB(p, st)
                    st = nxt

            # ================= collective: market_cols -> full market
            cin = dp.tile([C, PPC], F32)
            call = dp.tile([NCORE * C, PPC], F32)
            nc.sync.dma_start(cin[:], mcols[:])
            nc.gpsimd.collective_compute(
                "AllGather", mybir.AluOpType.bypass,
                replica_groups=[list(range(NCORE))],
                ins=[cin[:].opt()], outs=[call[:].opt()])
            call_v = call[:].rearrange("(blk c) j -> blk c j", blk=NCORE)
            nc.sync.dma_start(
                market[:].rearrange("c (blk j) -> c blk j", blk=NCORE),
                call_v.transpose([1, 0, 2]))

            # ================= phase 2/3: LN stats + GRU + head
            with (
                tc.tile_pool(name="w3p", bufs=2) as w3,
                tc.tile_pool(name="ps3", bufs=1, space="PSUM") as ps,
            ):
                nc.vector.tensor_copy(market_bf[:], market[:])
                # market sums (scaled by 1/222) -> (60,4) layout
                msq = w3.tile([C, B * D], F32, tag="msq")
                nc.scalar.square(msq[:], market[:])
                for src, dst in ((market, summc), (msq, sumsqc)):
                    pst = ps.tile([128, B * D + 8], F32, tag="pst", name="pst")
                    psum = pst[0:1, 0:B * D]
                    nc.tensor.matmul(psum, cst["c222v"][:], src[:],
                                     start=True, stop=True)
                    srow = w3.tile([1, B * D], F32, tag="srow")
                    nc.vector.tensor_copy(srow[:], psum)
                    # (1,240) row -> (60,4): per-b strided transpose matmuls
                    pmin = pst[0:D, B * D:B * D + 4]
                    srow_v = srow[:].rearrange("o (d b) -> o d b", b=4)
                    for b in range(4):
                        nc.tensor.matmul(pmin[:, b:b + 1],
                                         srow_v[0:1, :, b],
                                         onesb[0:1, 0:1],
                                         start=True, stop=True)
                    nc.vector.tensor_copy(dst[:], pmin)

                # LN statistics, in day-halves
                mu = w3.tile([D, N], F32, tag="mu")
                ms = w3.tile([D, N], F32, tag="ms")
                mu2 = w3.tile([D, N], F32, tag="mu2")
                var = w3.tile([D, N], F32, tag="var")
                std = w3.tile([D, N], F32, tag="std")
                rstd = w3.tile([D, N], F32, tag="rstd")
                dsplits = [(0, 32), (32, 60)]
                for g, (d0, d1) in enumerate(dsplits):
                    dd = slice(d0, d1)
                    nc.vector.tensor_tensor(
                        out=mu[dd, :].rearrange("p (s b) -> p s b", b=4),
                        in0=cst["xs"][dd, :].rearrange("p (s b) -> p s b", b=4),
                        in1=summc[dd, :].unsqueeze(1)
                            .broadcast_to([d1 - d0, SL, 4]),
                        op=AL.add)
                    nc.vector.tensor_tensor(
                        out=ms[dd, :].rearrange("p (s b) -> p s b", b=4),
                        in0=cst["xq"][dd, :].rearrange("p (s b) -> p s b", b=4),
                        in1=sumsqc[dd, :].unsqueeze(1)
                            .broadcast_to([d1 - d0, SL, 4]),
                        op=AL.add)
                    nc.vector.tensor_tensor(out=mu2[dd, :], in0=mu[dd, :],
                                            in1=mu[dd, :], op=AL.mult)
                    nc.vector.tensor_tensor(out=var[dd, :], in0=ms[dd, :],
                                            in1=mu2[dd, :], op=AL.subtract)
                    nc.scalar.activation(std[dd, :], var[dd, :], AF.Sqrt,
                                         bias=epsc[dd, :])
                    nc.vector.reciprocal(rstd[dd, :], std[dd, :])
                    nc.vector.tensor_copy(rstd2[dd, :], rstd[dd, :])
                    nc.vector.scalar_tensor_tensor(
                        out=rown2[dd, :], in0=rstd[dd, :], scalar=-1.0,
                        in1=mu[dd, :], op0=AL.mult, op1=AL.mult)
                rs_dram_a = dp.tile([D, N], BF16, name="rs_dram_a")
                rs_dram_b = dp.tile([D, N], BF16, name="rs_dram_b")
                for d0, d1 in dsplits:
                    dd = slice(d0, d1)
                    nc.sync.dma_start(rs_dram_a[dd, :], rstd2[dd, :])
                    nc.sync.dma_start(rs_dram_b[dd, :], rown2[dd, :])
                    nc.sync.dma_start(
                        rrowA[0:1, N * d0:N * d1],
                        rs_dram_a[dd, :].rearrange("p f -> () (p f)"))
                    nc.sync.dma_start(
                        rrowB[0:1, N * d0:N * d1],
                        rs_dram_b[dd, :].rearrange("p f -> () (p f)"))

                # ---- GRU over days (x-side prepped 4 days at a time)
                # h[k] = (t3_k, zh_k): h = t3 + zh, kept unsummed for WHH
                h = [[None, None], [None, None]]
                for k in range(2):
                    for q in range(2):
                        hz = w3.tile([H, N // 2], BF16, tag=f"h{k}{q}",
                                     name=f"h{k}{q}")
                        nc.vector.memset(hz[:], 0.0)
                        h[k][q] = hz
                # au1 rows 94:96 stay zero; row 96 = row2 (-mu*rstd)
                for _ in range(2):
                    t = w3.tile([97, 4 * N], BF16, tag="au1", name="au1_init")
                    nc.vector.memset(t[:], 0.0)

                def prep(d):
                    """prepare au0/au1 for days [d, d+4)."""
                    nd = min(4, D - d)
                    w = nd * N
                    psr = w3.tile([128, 4 * N], BF16, tag="psr", name="psr")
                    au0 = w3.tile([128, 4 * N], BF16, tag="au0", name="au0")
                    au1 = w3.tile([97, 4 * N], BF16, tag="au1", name="au1")
                    nc.gpsimd.partition_broadcast(
                        psr[:, 0:w], rrowA[0:1, N * d:N * d + w])
                    nc.sync.dma_start(au1[96:97, 0:w],
                                      rrowB[0:1, N * d:N * d + w])
                    nc.vector.tensor_tensor(out=au0[:, 0:w],
                                            in0=cst["xd0"][:, N * d:N * d + w],
                                            in1=psr[:, 0:w], op=AL.mult)
                    nc.vector.tensor_tensor(
                        out=au1[0:C, 0:w].rearrange(
                            "p (dd s b) -> p dd s b", dd=nd, b=4),
                        in0=market_bf[:, 4 * d:4 * (d + nd)].rearrange(
                            "c (dd b) -> c dd () b", dd=nd)
                            .broadcast_to([C, nd, SL, 4]),
                        in1=psr[0:C, 0:w].rearrange(
                            "p (dd s b) -> p dd s b", dd=nd, b=4),
                        op=AL.mult)
                    nc.vector.tensor_tensor(
                        out=au1[C:C + F1, 0:w],
                        in0=cst["xmx"][64:94, N * d:N * d + w],
                        in1=psr[C:C + F1, 0:w], op=AL.mult)
                    return au0, au1

                HF = N // 2  # column half: two independent chains
                cur = prep(0)
                nxt = None
                for d in range(D):
                    if d % 4 == 0 and d > 0:
                        cur = nxt
                    au0f, au1f = cur
                    au0 = au0f[:, ts(d % 4, N)]
                    au1 = au1f[:, ts(d % 4, N)]
                    RZ, XN = [None, None], [None, None]
                    for k in range(2):
                        RZ[k] = ps.tile([128, 2 * HF], F32, tag=f"RZ{k}",
                                        name=f"RZ{k}")
                        XNt = ps.tile([128, 512], F32, tag=f"XN{k}",
                                        name=f"XN{k}")
                        XN[k] = XNt[:, 0:HF]
                    HNp = ps.tile([128, 2 * HF], F32, tag="HNp", name="HNp")
                    # accumulation groups strictly sequenced per psum bank
                    for k in range(2):
                        cc = ts(k, HF)
                        for gi in range(2):  # r, z gates -> RZ[k] halves
                            reg = RZ[k][:, ts(gi, HF)]
                            nc.tensor.matmul(reg, cst["LT0"][:, ts(gi, 128)],
                                             au0[:, cc], start=True,
                                             stop=False)
                            nc.tensor.matmul(reg, cst["LT1"][:, ts(gi, 128)],
                                             au1[:, cc], start=False,
                                             stop=False)
                            nc.tensor.matmul(reg, cst["WHH"][:, ts(gi, 128)],
                                             h[k][1][:], start=False,
                                             stop=False)
                            nc.tensor.matmul(reg, cst["WHH"][:, ts(gi, 128)],
                                             h[k][0][:], start=False,
                                             stop=True)
                        nc.tensor.matmul(XN[k], cst["LT0"][:, ts(2, 128)],
                                         au0[:, cc], start=True, stop=False)
                        nc.tensor.matmul(XN[k], cst["LT1"][:, ts(2, 128)],
                                         au1[:, cc], start=False, stop=False)
                        hreg = HNp[:, ts(k, HF)]
                        nc.tensor.matmul(hreg, cst["WHH"][:, ts(2, 128)],
                                         h[k][1][:], start=True, stop=False)
                        nc.tensor.matmul(hreg, cst["WHH"][:, ts(2, 128)],
                                         h[k][0][:], start=False, stop=True)
                    r_sb, z_sb, t1, zc, hs, zh, n_sb, t3 = (
                        [None, None] for _ in range(8))
                    for k in range(2):
                        r_sb[k] = w3.tile([H, HF], BF16, tag=f"r{k}",
                                          name=f"r{k}")
                        nc.scalar.activation(r_sb[k][:], RZ[k][:, 0:HF],
                                             AF.Sigmoid,
                                             bias=cst["BBC"][:, 0:1])
                    for k in range(2):
                        z_sb[k] = w3.tile([H, HF], BF16, tag=f"z{k}",
                                          name=f"z{k}")
                        nc.scalar.activation(z_sb[k][:], RZ[k][:, HF:2 * HF],
                                             AF.Sigmoid,
                                             bias=cst["BBC"][:, 1:2])
                    for k in range(2):
                        t1[k] = w3.tile([H, HF], BF16, tag=f"t1{k}",
                                        name=f"t1{k}")
                        nc.vector.scalar_tensor_tensor(
                            out=t1[k][:], in0=HNp[:, ts(k, HF)],
                            scalar=cst["bhh_n"][:], in1=r_sb[k][:],
                            op0=AL.add, op1=AL.mult)
                        nc.tensor.matmul(XN[k], cst["identb"][:],
                                         t1[k][:], start=False, stop=True)
                        zc[k] = w3.tile([H, HF], BF16, tag=f"zc{k}",
                                        name=f"zc{k}")
                        nc.gpsimd.tensor_scalar(out=zc[k][:], in0=z_sb[k][:],
                                                scalar1=-1.0, scalar2=1.0,
                                                op0=AL.mult, op1=AL.add)
                        hs[k] = w3.tile([H, HF], BF16, tag=f"hs{k}",
                                        name=f"hs{k}")
                        nc.vector.tensor_tensor(out=hs[k][:], in0=h[k][0][:],
                                                in1=h[k][1][:], op=AL.add)
                        zh[k] = w3.tile([H, HF], BF16, tag=f"zh{k}",
                                        name=f"zh{k}")
                        nc.vector.tensor_tensor(out=zh[k][:], in0=z_sb[k][:],
                                                in1=hs[k][:], op=AL.mult)
                    if d + 1 < D and (d + 1) % 4 == 0:
                        nxt = prep(d + 1)
                    for k in range(2):
                        n_sb[k] = w3.tile([H, HF], BF16, tag=f"n{k}",
                                          name=f"n{k}")
                        nc.scalar.activation(n_sb[k][:], XN[k], AF.Tanh,
                                             bias=cst["BBC"][:, 2:3])
                        t3[k] = w3.tile([H, HF], BF16, tag=f"t3{k}",
                                        name=f"t3{k}")
                        nc.vector.tensor_tensor(out=t3[k][:], in0=n_sb[k][:],
                                                in1=zc[k][:], op=AL.mult)
                    h = [(t3[0], zh[0]), (t3[1], zh[1])]

                # ---- head
                phdt = ps.tile([128, 2 * HF], F32, tag="RZ0")
                phd = phdt[0:C, :]
                for k in range(2):
                    nc.tensor.matmul(phd[:, ts(k, HF)], cst["W1"][:],
                                     h[k][0][:], start=True, stop=False)
                    nc.tensor.matmul(phd[:, ts(k, HF)], cst["W1"][:],
                                     h[k][1][:], start=False, stop=True)
                hid = w3.tile([C, N], BF16, tag="hid")
                nc.scalar.activation(hid[:], phd, AF.Relu,
                                     bias=cst["B1"][:])
                psot = ps.tile([128, 2 * HF], F32, tag="HNp")
                pso = psot[0:1, 0:N]
                nc.tensor.matmul(pso, cst["W2"][:], hid[:],
                                 start=True, stop=True)
                yo = w3.tile([1, N], F32, tag="yo")
                nc.scalar.activation(yo[:], pso, AF.Identity,
                                     bias=cst["B2"][0:1, 0:1])
                nc.sync.dma_start(yout, yo[:])

    nc.compile()
    return nc


_NC_CACHE = None


def kernel(**inputs):
    global _NC_CACHE
    per_core = host_prep(**inputs)
    if _NC_CACHE is None:
        _NC_CACHE = build_program()
    nc = _NC_CACHE
    in_maps = [{k: pc[k] for k in INPUT_SPECS} for pc in per_core]
    res = run_bass_kernel_spmd(nc, in_maps, list(range(NCORE)))
    out = np.zeros((B, S, OUT), np.float32)
    for i in range(NCORE):
        out[:, SL * i:SL * (i + 1), 0] = (
            res.results[i]["yout"].reshape(SL, B).T)
    return out


# revision 18
# speedup vs baseline: 1.1511x; 1.0283x over previous
"""Trainium2 Bass kernel for nn_GRUWithAttentionModel (B=4,D=60,S=512,F=158,C=64,H=128).

Sharding: phase 1 (per-day attention -> market vector) is sharded over the 240
(day,batch) pairs, 30 per core; the tiny (64,240) market matrix is AllGathered;
phase 2/3 (layernorm + GRU over days + head) is sharded over the 512 stocks,
64 per core (256 sequences/core).

Key ideas over the naive formulation:
- queries AND keys are compacted host-side: masked stocks contribute exactly
  0 to the market vector (query mask u=0; key mask exp(-2.5e8)=0), so each
  pair keeps only the first 384 mask-sorted stocks (max unmasked ~300) and
  Q/K/V all read the same compacted x tile.
- ctx is computed in [stock, C] orientation directly (no transposes); the
  softmax denominator rides along as a ones column of the V tiles.
- the LN rank-1 term (-mu*rstd x A) is folded into the LT1 matmul via an
  extra row of au1 (kills 6 rank-1 matmuls per day).
- GRU state stays split as h = t3 + zh (keeps the serial per-day dependency
  chain short); the n-gate r*(HN+bhh) runs as a cheap all-SBUF bf16 multiply
  against an off-chain HN+bhh evacuation.
- x-side prep is done 4 days at a time from flat rstd/-mu*rstd rows.

All bulk tensors are staged bf16 (inputs ~N(0,1), tolerance 2e-2); matmul
accumulation stays fp32 in PSUM.

Self-contained: call kernel(**inputs) with the full reference.setup_inputs()
arrays; returns the full (4,512,1) output.
"""
import numpy as np
import ml_dtypes

from concourse import bacc, tile, mybir
from concourse.bass import ts
from concourse.bass_utils import run_bass_kernel_spmd

F32 = mybir.dt.float32
BF16 = mybir.dt.bfloat16
BF = ml_dtypes.bfloat16

B, D, S, F, C, H, OUT = 4, 60, 512, 158, 64, 128, 1
LN_EPS = 1e-5
NCORE = 8
PPC = (B * D) // NCORE      # 30 pairs/core, pair p = d*4+b (d-major)
SL = S // NCORE             # 64 stocks/core
N = B * SL                  # 256 sequences/core
FC = F + C                  # 222
G3 = 3 * H                  # 384
F1 = F - 128                # 30 leftover feature rows
FA1 = F1 + 1                # +1 mask row (carries QKV biases for live stocks)
NK = 384                    # compacted (mask-sorted) stocks per pair
NKC = NK // 128             # 3 chunks


# ---------------------------------------------------------------- host prep
def host_prep(x, feature_mask, wq, bq, wk, bk, wv, bv, ln_g, ln_b,
              w_ih, w_hh, b_ih, b_hh, w1, b1, w2, b2):
    f32 = np.float32
    x = np.asarray(x, f32)
    x_att = x.transpose(1, 0, 2, 3).reshape(B * D, S, F)          # p = d*4+b
    mask_p = np.asarray(feature_mask, f32).transpose(1, 0, 2).reshape(B * D, S)
    denom = np.maximum(mask_p.sum(1), 1.0).astype(f32)
    u = (mask_p / denom[:, None]).astype(f32)

    # mask-sorted stock order per pair: unmasked first, truncate to NK
    order = np.argsort(~mask_p.astype(bool), axis=1, kind='stable')[:, :NK]
    assert mask_p.sum(1).max() <= NK
    x_key = np.take_along_axis(x_att, order[:, :, None], axis=1)  # (BD,NK,F)
    mask_key = np.take_along_axis(mask_p, order, axis=1)          # (BD,NK)
    u_key = np.take_along_axis(u, order, axis=1)                  # (BD,NK)
    mneg3 = ((1.0 - mask_key) * np.float32(-2e9)).astype(f32)

    wp = (np.asarray(w_ih, f32) * np.asarray(ln_g, f32)[None, :])
    A = wp.sum(1).astype(f32)
    Bb = (np.asarray(w_ih, f32) @ np.asarray(ln_b, f32) + np.asarray(b_ih, f32))
    Bb = Bb.copy()
    Bb[:2 * H] += np.asarray(b_hh, f32)[:2 * H]
    # LT1 rows: [market 64 | x-feats 128:158 (30) | pad 2 | A @96]
    # (Bb biases are applied at the activations, not in the matmul)
    LT0 = np.ascontiguousarray(wp.T[:128]).astype(BF)             # (128,384)
    LT1 = np.ascontiguousarray(np.concatenate(
        [wp.T[F:FC], wp.T[128:F], np.zeros((2, G3), f32), A[None]],
        0)).astype(BF)                                            # (97,384)
    BBC = np.ascontiguousarray(Bb.reshape(3, 128).T.copy())       # (128,3)

    bqkv = np.stack([np.asarray(bq, f32), np.asarray(bk, f32),
                     np.asarray(bv, f32)], 0).reshape(1, 192)    # [bq|bk|bv]

    per_core = []
    for i in range(NCORE):
        pi = list(range(i, B * D, NCORE))  # strided: local j <-> global 8j+i
        sl = slice(SL * i, SL * (i + 1))
        xs_sl = x[:, :, sl, :]
        xsum = (xs_sl.sum(-1) / np.float32(FC)).astype(f32)
        xsq = ((xs_sl * xs_sl).sum(-1) / np.float32(FC)).astype(f32)
        # compacted attention side: (158, PPC*NK), shared by Q, K and V
        xkey_T = np.ascontiguousarray(
            x_key[pi].transpose(2, 0, 1).reshape(F, PPC * NK)).astype(BF)
        # seq side: (158, D*N): col = d*256 + seq
        xseq_T = np.ascontiguousarray(
            xs_sl.transpose(3, 1, 2, 0).reshape(F, D * N)).astype(BF)
        # xmx rows 0:31 = [x-key 128:158, maskrow] (cols 0:PPC*NK);
        # rows 31:61 = x-seq 128:158 (cols 0:D*N)
        xk1 = np.concatenate(
            [xkey_T[128:F], mask_key[pi].reshape(1, PPC * NK).astype(BF)], 0)
        xmx = np.zeros((94, PPC * S), BF)
        xmx[0:FA1, :PPC * NK] = xk1
        xmx[64:94] = xseq_T[128:F]
        per_core.append(dict(
            xk0=xkey_T[0:128], xmx=np.ascontiguousarray(xmx),
            xd0=xseq_T[0:128],
            uT=np.ascontiguousarray(
                u_key[pi].reshape(PPC, NKC, 128).transpose(2, 0, 1)
                .reshape(128, PPC * NKC)),
            mnegT=np.ascontiguousarray(
                mneg3[pi].reshape(PPC, NKC, 128).transpose(2, 0, 1)
                .reshape(128, PPC * NKC)),
            xs=np.ascontiguousarray(xsum.transpose(1, 2, 0).reshape(D, N)),
            xq=np.ascontiguousarray(xsq.transpose(1, 2, 0).reshape(D, N)),
            LT0=LT0, LT1=LT1, BBC=BBC,
            WHH=np.ascontiguousarray(np.asarray(w_hh, f32).T).astype(BF),
            bhh_n=np.ascontiguousarray(np.asarray(b_hh, f32)[2 * H:][:, None]),
            WQKV0=np.ascontiguousarray(np.concatenate(
                [wq[:128], wk[:128], wv[:128]], 1).astype(f32)).astype(BF),
            WQKV1=np.ascontiguousarray(np.concatenate(
                [np.concatenate([wq[128:], wk[128:], wv[128:]], 1), bqkv],
                0).astype(f32)).astype(BF),                       # (31,192)
            W1=np.ascontiguousarray(np.asarray(w1, f32)).astype(BF),
            B1=np.ascontiguousarray(np.asarray(b1, f32)[:, None]),
            W2=np.ascontiguousarray(np.asarray(w2, f32)).astype(BF),
            c222v=np.full((C, 1), 1.0 / FC, f32),
            B2=np.ascontiguousarray(np.asarray(b2, f32)[None, :]),
            identb=np.eye(128, dtype=f32).astype(BF),
        ))
    return per_core


INPUT_SPECS = dict(
    xk0=((128, PPC * NK), BF16), xmx=((94, PPC * S), BF16),
    xd0=((128, D * N), BF16),
    uT=((128, PPC * NKC), F32), mnegT=((128, PPC * NKC), F32),
    xs=((D, N), F32), xq=((D, N), F32),
    LT0=((128, G3), BF16), LT1=((97, G3), BF16),
    BBC=((128, 3), F32),
    WHH=((H, G3), BF16), bhh_n=((H, 1), F32),
    WQKV0=((128, 192), BF16), WQKV1=((FA1, 192), BF16),
    W1=((H, C), BF16), B1=((C, 1), F32), W2=((C, 1), BF16), B2=((1, 1), F32),
    c222v=((C, 1), F32), identb=((128, 128), BF16),
)


# ---------------------------------------------------------------- program
def build_program():
    nc = bacc.Bacc("TRN2", target_bir_lowering=False, debug=False,
                   num_devices=NCORE)
    dram = {k: nc.dram_tensor(k, list(shp), dt, kind="ExternalInput").ap()
            for k, (shp, dt) in INPUT_SPECS.items()}
    yout = nc.dram_tensor("yout", [1, N], F32, kind="ExternalOutput").ap()
    AL = mybir.AluOpType
    AF = mybir.ActivationFunctionType

    with tile.TileContext(nc) as tc:
        with (
            nc.allow_low_precision(reason="bf16 staging within 2e-2 tolerance"),
            tc.tile_pool(name="const", bufs=1) as cp,
            tc.tile_pool(name="dram", bufs=1, space="DRAM") as dp,
        ):
            # ---- persistent tiles (weights + preloaded activations)
            cst = {}
            for k in INPUT_SPECS:
                shp, dt = INPUT_SPECS[k]
                cst[k] = cp.tile(list(shp), dt, tag=k, name=k)
            for k in ("WQKV0", "WQKV1", "mnegT", "uT"):
                nc.sync.dma_start(cst[k][:], dram[k])
            bounds = [0, 1, 3, 6, 10, 15, 21, 30]  # pair-index chunk edges
            for ci in range(len(bounds) - 1):
                klo, khi = bounds[ci] * NK, bounds[ci + 1] * NK
                nc.sync.dma_start(cst["xk0"][:, klo:khi],
                                  dram["xk0"][:, klo:khi])
                nc.sync.dma_start(cst["xmx"][0:FA1, klo:khi],
                                  dram["xmx"][0:FA1, klo:khi])
            for k in ("xd0", "xs", "xq", "LT0", "LT1", "WHH", "bhh_n",
                      "W1", "B1", "W2", "B2", "identb", "c222v", "BBC"):
                nc.sync.dma_start(cst[k][:], dram[k])
            nc.sync.dma_start(cst["xmx"][64:94, :], dram["xmx"][64:94, :])
            onesb = cp.tile([1, 1], F32, tag="onesb")
            nc.vector.memset(onesb[:], 1.0)
            epsc = cp.tile([D, 1], F32, tag="epsc")
            nc.vector.memset(epsc[:], LN_EPS)
            mcols = cp.tile([C, PPC], F32, tag="mcols")
            market = cp.tile([C, B * D], F32, tag="market")
            market_bf = cp.tile([C, B * D], BF16, tag="market_bf")
            summc = cp.tile([D, 4], F32, tag="summc")
            sumsqc = cp.tile([D, 4], F32, tag="sumsqc")
            rstd2 = cp.tile([D, N], BF16, tag="rstd2")
            rown2 = cp.tile([D, N], BF16, tag="rown2")
            rrowA = cp.tile([1, D * N], BF16, tag="rrowA")
            rrowB = cp.tile([1, D * N], BF16, tag="rrowB")

            # ================= phase 1: attention -> market columns
            with (
                tc.tile_pool(name="w1p", bufs=2) as wp,
                tc.tile_pool(name="ps1", bufs=1, space="PSUM") as ps,
            ):
                # pre-set the ones column of the V tiles (both pool bufs)
                for _ in range(2):
                    t = wp.tile([128, NKC, C + 1], BF16, tag="ve",
                                name="ve_init")
                    for kc in range(NKC):
                        nc.gpsimd.memset(t[:, kc, C:C + 1], 1.0)

                def stageA(p):
                    xk0 = cst["xk0"][:, ts(p, NK)]
                    xk1 = cst["xmx"][0:FA1, ts(p, NK)]
                    qkt = ps.tile([128, 512], F32, tag="qkt", name="qkt")
                    pq = qkt[0:64, 0:NK]
                    pk = qkt[64:128, 0:NK]
                    nc.tensor.matmul(pq, cst["WQKV0"][:, ts(0, C)],
                                     xk0, start=True, stop=False)
                    nc.tensor.matmul(pq, cst["WQKV1"][:, ts(0, C)],
                                     xk1, start=False, stop=True)
                    nc.tensor.matmul(pk, cst["WQKV0"][:, ts(1, C)],
                                     xk0, start=True, stop=False)
                    nc.tensor.matmul(pk, cst["WQKV1"][:, ts(1, C)],
                                     xk1, start=False, stop=True)
                    q_sb = wp.tile([C, NK], BF16, tag="q", name="q_sb")
                    nc.vector.tensor_copy(q_sb[:], pq)
                    k_sb = wp.tile([C, NK], BF16, tag="k", name="k_sb")
                    nc.vector.tensor_copy(k_sb[:], pk)
                    # V chunks [128 stocks, C] (+ ones col set at pool init)
                    pv = ps.tile([128, NKC, C], F32, tag="pv", name="pv")
                    for kc in range(NKC):
                        nc.tensor.matmul(pv[:, kc, :], xk0[:, ts(kc, 128)],
                                         cst["WQKV0"][:, ts(2, C)],
                                         start=True, stop=False)
                        nc.tensor.matmul(pv[:, kc, :], xk1[:, ts(kc, 128)],
                                         cst["WQKV1"][:, ts(2, C)],
                                         start=False, stop=True)
                    ve = wp.tile([128, NKC, C + 1], BF16, tag="ve", name="ve")
                    nc.vector.tensor_copy(ve[:, :, 0:C], pv[:])
                    return q_sb, k_sb, ve

                def stageB(p, st):
                    q_sb, k_sb, ve = st
                    pss, eT = [], []
                    for c in range(NKC):
                        pt = ps.tile([128, NK], F32, tag=f"ss{c}", name="pss")
                        nc.tensor.matmul(pt[:], k_sb[:, ts(c, 128)],
                                         q_sb[:], start=True, stop=True)
                        pss.append(pt)
                    for c in range(NKC):
                        et = wp.tile([128, NK], BF16, tag=f"eT{c}", name="et")
                        nc.scalar.activation(
                            et[:], pss[c][:], AF.Exp, scale=0.125,
                            bias=cst["mnegT"][:, NKC * p + c:NKC * p + c + 1])
                        eT.append(et)
                    # ctx in [stock, C(+denom)] orientation, NKC stock chunks
                    cxm = ps.tile([128, NKC, C + 1], F32, tag="cxm",
                                  name="cxm")
                    for sc in range(NKC):
                        for tc_ in range(NKC):
                            nc.tensor.matmul(cxm[:, sc, :],
                                             eT[tc_][:, ts(sc, 128)],
                                             ve[:, tc_, :],
                                             start=(tc_ == 0),
                                             stop=(tc_ == NKC - 1))
                    rr = wp.tile([128, NKC], F32, tag="rr", name="rr")
                    nc.vector.reciprocal(rr[:], cxm[:, :, C])
                    gT = wp.tile([128, NKC], BF16, tag="gT", name="gT")
                    nc.vector.tensor_tensor(
                        out=gT[:], in0=cst["uT"][:, NKC * p:NKC * (p + 1)],
                        in1=rr[:], op=AL.mult)
                    cx2 = wp.tile([128, NKC, C], BF16, tag="cx2", name="cx2")
                    nc.vector.tensor_copy(cx2[:], cxm[:, :, 0:C])
                    psm = ps.tile([C, 1], F32, tag="psm", name="psm")
                    for sc in range(NKC):
                        nc.tensor.matmul(psm[:], cx2[:, sc, :],
                                         gT[:, sc:sc + 1],
                                         start=(sc == 0), stop=(sc == NKC - 1))
                    nc.vector.tensor_copy(mcols[:, p:p + 1], psm[:])

                cinA = dp.tile([C, 16], F32)
                callA = dp.tile([NCORE * C, 16], F32)
                cinB = dp.tile([C, PPC - 16], F32)
                callB = dp.tile([NCORE * C, PPC - 16], F32)

                def redist(call, j0, j1):
                    # market col (8j+blk) <- call[blk*C + c, j]
                    cv = call[:].rearrange("(blk c) j -> blk c j", blk=NCORE)
                    nc.sync.dma_start(
                        market[:, 8 * j0:8 * j1].rearrange(
                            "c (j blk) -> c j blk", blk=NCORE),
                        cv.transpose([1, 2, 0]))

                st = stageA(0)
                for p in range(PPC):
                    nxt = stageA(p + 1) if p + 1 < PPC else None
                    stageB(p, st)
                    st = nxt
                    if p == 15:
                        nc.sync.dma_start(cinA[:], mcols[:, 0:16])
                        nc.gpsimd.collective_compute(
                            "AllGather", mybir.AluOpType.bypass,
                            replica_groups=[list(range(NCORE))],
                            ins=[cinA[:].opt()], outs=[callA[:].opt()])
                        redist(callA, 0, 16)
                    if p == PPC - 1:
                        nc.sync.dma_start(cinB[:], mcols[:, 16:PPC])
                        nc.gpsimd.collective_compute(
                            "AllGather", mybir.AluOpType.bypass,
                            replica_groups=[list(range(NCORE))],
                            ins=[cinB[:].opt()], outs=[callB[:].opt()])

            # ================= phase 2/3: LN stats + GRU + head
            with (
                tc.tile_pool(name="w3p", bufs=2) as w3,
                tc.tile_pool(name="ps3", bufs=1, space="PSUM") as ps,
            ):
                mu = w3.tile([D, N], F32, tag="mu")
                ms = w3.tile([D, N], F32, tag="ms")
                mu2 = w3.tile([D, N], F32, tag="mu2")
                var = w3.tile([D, N], F32, tag="var")
                std = w3.tile([D, N], F32, tag="std")
                rstd = w3.tile([D, N], F32, tag="rstd")
                msq = w3.tile([C, B * D], F32, tag="msq")
                rs_dram_a = dp.tile([D, N], BF16, name="rs_dram_a")
                rs_dram_b = dp.tile([D, N], BF16, name="rs_dram_b")

                def stats_ln_half(d0, d1):
                    c0, c1 = 4 * d0, 4 * d1
                    nd = d1 - d0
                    nc.vector.tensor_copy(market_bf[:, c0:c1],
                                          market[:, c0:c1])
                    nc.scalar.square(msq[:, c0:c1], market[:, c0:c1])
                    pst = ps.tile([128, 256], F32, tag="pst", name="pst")
                    for src_, dst in ((market, summc), (msq, sumsqc)):
                        psum = pst[0:1, 0:c1 - c0]
                        nc.tensor.matmul(psum, cst["c222v"][:],
                                         src_[:, c0:c1],
                                         start=True, stop=True)
                        srow = w3.tile([1, B * D], F32, tag="srow")
                        nc.vector.tensor_copy(srow[0:1, 0:c1 - c0], psum)
                        pmin = pst[d0:d1, 248:252]
                        srow_v = srow[0:1, 0:c1 - c0].rearrange(
                            "o (d b) -> o d b", b=4)
                        for b in range(4):
                            nc.tensor.matmul(pmin[:, b:b + 1],
                                             srow_v[0:1, :, b],
                                             onesb[0:1, 0:1],
                                             start=True, stop=True)
                        nc.vector.tensor_copy(dst[d0:d1, :], pmin)
                    dd = slice(d0, d1)
                    nc.vector.tensor_tensor(
                        out=mu[dd, :].rearrange("p (s b) -> p s b", b=4),
                        in0=cst["xs"][dd, :].rearrange("p (s b) -> p s b", b=4),
                        in1=summc[dd, :].unsqueeze(1)
                            .broadcast_to([d1 - d0, SL, 4]),
                        op=AL.add)
                    nc.vector.tensor_tensor(
                        out=ms[dd, :].rearrange("p (s b) -> p s b", b=4),
                        in0=cst["xq"][dd, :].rearrange("p (s b) -> p s b", b=4),
                        in1=sumsqc[dd, :].unsqueeze(1)
                            .broadcast_to([d1 - d0, SL, 4]),
                        op=AL.add)
                    nc.vector.tensor_tensor(out=mu2[dd, :], in0=mu[dd, :],
                                            in1=mu[dd, :], op=AL.mult)
                    nc.vector.tensor_tensor(out=var[dd, :], in0=ms[dd, :],
                                            in1=mu2[dd, :], op=AL.subtract)
                    nc.scalar.activation(std[dd, :], var[dd, :], AF.Sqrt,
                                         bias=epsc[dd, :])
                    nc.vector.reciprocal(rstd[dd, :], std[dd, :])
                    nc.vector.tensor_copy(rstd2[dd, :], rstd[dd, :])
                    nc.vector.scalar_tensor_tensor(
                        out=rown2[dd, :], in0=rstd[dd, :], scalar=-1.0,
                        in1=mu[dd, :], op0=AL.mult, op1=AL.mult)
                    nc.sync.dma_start(rs_dram_a[dd, :], rstd2[dd, :])
                    nc.sync.dma_start(rs_dram_b[dd, :], rown2[dd, :])
                    nc.sync.dma_start(
                        rrowA[0:1, N * d0:N * d1],
                        rs_dram_a[dd, :].rearrange("p f -> () (p f)"))
                    nc.sync.dma_start(
                        rrowB[0:1, N * d0:N * d1],
                        rs_dram_b[dd, :].rearrange("p f -> () (p f)"))

                stats_ln_half(0, 32)

                # ---- GRU over days (x-side prepped 4 days at a time)
                # h[k] = (t3_k, zh_k): h = t3 + zh, kept unsummed for WHH
                h = [[None, None], [None, None]]
                for k in range(2):
                    for q in range(2):
                        hz = w3.tile([H, N // 2], BF16, tag=f"h{k}{q}",
                                     name=f"h{k}{q}")
                        nc.vector.memset(hz[:], 0.0)
                        h[k][q] = hz
                # au1 rows 94:96 stay zero; row 96 = row2 (-mu*rstd)
                for _ in range(2):
                    t = w3.tile([97, 4 * N], BF16, tag="au1", name="au1_init")
                    nc.vector.memset(t[:], 0.0)

                def prep(d):
                    """prepare au0/au1 for days [d, d+4)."""
                    nd = min(4, D - d)
                    w = nd * N
                    psr = w3.tile([128, 4 * N], BF16, tag="psr", name="psr")
                    au0 = w3.tile([128, 4 * N], BF16, tag="au0", name="au0")
                    au1 = w3.tile([97, 4 * N], BF16, tag="au1", name="au1")
                    nc.gpsimd.partition_broadcast(
                        psr[:, 0:w], rrowA[0:1, N * d:N * d + w])
                    nc.sync.dma_start(au1[96:97, 0:w],
                                      rrowB[0:1, N * d:N * d + w])
                    nc.vector.tensor_tensor(out=au0[:, 0:w],
                                            in0=cst["xd0"][:, N * d:N * d + w],
                                            in1=psr[:, 0:w], op=AL.mult)
                    nc.vector.tensor_tensor(
                        out=au1[0:C, 0:w].rearrange(
                            "p (dd s b) -> p dd s b", dd=nd, b=4),
                        in0=market_bf[:, 4 * d:4 * (d + nd)].rearrange(
                            "c (dd b) -> c dd () b", dd=nd)
                            .broadcast_to([C, nd, SL, 4]),
                        in1=psr[0:C, 0:w].rearrange(
                            "p (dd s b) -> p dd s b", dd=nd, b=4),
                        op=AL.mult)
                    nc.vector.tensor_tensor(
                        out=au1[C:C + F1, 0:w],
                        in0=cst["xmx"][64:94, N * d:N * d + w],
                        in1=psr[C:C + F1, 0:w], op=AL.mult)
                    return au0, au1

                HF = N // 2  # column half: two independent chains
                cur = prep(0)
                nxt = None
                for d in range(D):
                    if d == 10:
                        redist(callB, 16, PPC)
                        stats_ln_half(32, 60)
                    if d % 4 == 0 and d > 0:
                        cur = nxt
                    au0f, au1f = cur
                    au0 = au0f[:, ts(d % 4, N)]
                    au1 = au1f[:, ts(d % 4, N)]
                    RZ, XN = [None, None], [None, None]
                    for k in range(2):
                        RZ[k] = ps.tile([128, 2 * HF], F32, tag=f"RZ{k}",
                                        name=f"RZ{k}")
                        XNt = ps.tile([128, 512], F32, tag=f"XN{k}",
                                        name=f"XN{k}")
                        XN[k] = XNt[:, 0:HF]
                    HNp = ps.tile([128, 2 * HF], F32, tag="HNp", name="HNp")
                    # accumulation groups strictly sequenced per psum bank
                    for k in range(2):
                        cc = ts(k, HF)
                        for gi in range(2):  # r, z gates -> RZ[k] halves
                            reg = RZ[k][:, ts(gi, HF)]
                            nc.tensor.matmul(reg, cst["LT0"][:, ts(gi, 128)],
                                             au0[:, cc], start=True,
                                             stop=False)
                            nc.tensor.matmul(reg, cst["LT1"][:, ts(gi, 128)],
                                             au1[:, cc], start=False,
                                             stop=False)
                            nc.tensor.matmul(reg, cst["WHH"][:, ts(gi, 128)],
                                             h[k][1][:], start=False,
                                             stop=False)
                            nc.tensor.matmul(reg, cst["WHH"][:, ts(gi, 128)],
                                             h[k][0][:], start=False,
                                             stop=True)
                        nc.tensor.matmul(XN[k], cst["LT0"][:, ts(2, 128)],
                                         au0[:, cc], start=True, stop=False)
                        nc.tensor.matmul(XN[k], cst["LT1"][:, ts(2, 128)],
                                         au1[:, cc], start=False, stop=False)
                        hreg = HNp[:, ts(k, HF)]
                        nc.tensor.matmul(hreg, cst["WHH"][:, ts(2, 128)],
                                         h[k][1][:], start=True, stop=False)
                        nc.tensor.matmul(hreg, cst["WHH"][:, ts(2, 128)],
                                         h[k][0][:], start=False, stop=True)
                    r_sb, z_sb, t1, zc, hs, zh, n_sb, t3 = (
                        [None, None] for _ in range(8))
                    for k in range(2):
                        r_sb[k] = w3.tile([H, HF], BF16, tag=f"r{k}",
                                          name=f"r{k}")
                        nc.scalar.activation(r_sb[k][:], RZ[k][:, 0:HF],
                                             AF.Sigmoid,
                                             bias=cst["BBC"][:, 0:1])
                    for k in range(2):
                        z_sb[k] = w3.tile([H, HF], BF16, tag=f"z{k}",
                                          name=f"z{k}")
                        nc.scalar.activation(z_sb[k][:], RZ[k][:, HF:2 * HF],
                                             AF.Sigmoid,
                                             bias=cst["BBC"][:, 1:2])
                    for k in range(2):
                        t1[k] = w3.tile([H, HF], BF16, tag=f"t1{k}",
                                        name=f"t1{k}")
                        nc.vector.scalar_tensor_tensor(
                            out=t1[k][:], in0=HNp[:, ts(k, HF)],
                            scalar=cst["bhh_n"][:], in1=r_sb[k][:],
                            op0=AL.add, op1=AL.mult)
                        nc.tensor.matmul(XN[k], cst["identb"][:],
                                         t1[k][:], start=False, stop=True)
                        zc[k] = w3.tile([H, HF], BF16, tag=f"zc{k}",
                                        name=f"zc{k}")
                        nc.gpsimd.tensor_scalar(out=zc[k][:], in0=z_sb[k][:],
                                                scalar1=-1.0, scalar2=1.0,
                                                op0=AL.mult, op1=AL.add)
                        hs[k] = w3.tile([H, HF], BF16, tag=f"hs{k}",
                                        name=f"hs{k}")
                        nc.vector.tensor_tensor(out=hs[k][:], in0=h[k][0][:],
                                                in1=h[k][1][:], op=AL.add)
                        zh[k] = w3.tile([H, HF], BF16, tag=f"zh{k}",
                                        name=f"zh{k}")
                        nc.vector.tensor_tensor(out=zh[k][:], in0=z_sb[k][:],
                                                in1=hs[k][:], op=AL.mult)
                    if d + 1 < D and (d + 1) % 4 == 0:
                        nxt = prep(d + 1)
                    for k in range(2):
                        n_sb[k] = w3.tile([H, HF], BF16, tag=f"n{k}",
                                          name=f"n{k}")
                        nc.scalar.activation(n_sb[k][:], XN[k], AF.Tanh,
                                             bias=cst["BBC"][:, 2:3])
                        t3[k] = w3.tile([H, HF], BF16, tag=f"t3{k}",
                                        name=f"t3{k}")
                        nc.vector.tensor_tensor(out=t3[k][:], in0=n_sb[k][:],
                                                in1=zc[k][:], op=AL.mult)
                    h = [(t3[0], zh[0]), (t3[1], zh[1])]

                # ---- head
                phdt = ps.tile([128, 2 * HF], F32, tag="RZ0")
                phd = phdt[0:C, :]
                for k in range(2):
                    nc.tensor.matmul(phd[:, ts(k, HF)], cst["W1"][:],
                                     h[k][0][:], start=True, stop=False)
                    nc.tensor.matmul(phd[:, ts(k, HF)], cst["W1"][:],
                                     h[k][1][:], start=False, stop=True)
                hid = w3.tile([C, N], BF16, tag="hid")
                nc.scalar.activation(hid[:], phd, AF.Relu,
                                     bias=cst["B1"][:])
                psot = ps.tile([128, 2 * HF], F32, tag="HNp")
                pso = psot[0:1, 0:N]
                nc.tensor.matmul(pso, cst["W2"][:], hid[:],
                                 start=True, stop=True)
                yo = w3.tile([1, N], F32, tag="yo")
                nc.scalar.activation(yo[:], pso, AF.Identity,
                                     bias=cst["B2"][0:1, 0:1])
                nc.sync.dma_start(yout, yo[:])

    nc.compile()
    return nc


_NC_CACHE = None


def kernel(**inputs):
    global _NC_CACHE
    per_core = host_prep(**inputs)
    if _NC_CACHE is None:
        _NC_CACHE = build_program()
    nc = _NC_CACHE
    in_maps = [{k: pc[k] for k in INPUT_SPECS} for pc in per_core]
    res = run_bass_kernel_spmd(nc, in_maps, list(range(NCORE)))
    out = np.zeros((B, S, OUT), np.float32)
    for i in range(NCORE):
        out[:, SL * i:SL * (i + 1), 0] = (
            res.results[i]["yout"].reshape(SL, B).T)
    return out
